# revision 1
# baseline (speedup 1.0000x reference)
"""2-layer dense GCN on 8 Trainium2 NeuronCores.

Reference computation (all fp32):
    H0 = relu((A_norm @ X) @ W0)
    H1 = relu((A_norm @ H0) @ W1)
A_norm: [16384, 16384], X: [16384, 128], W0/W1: [128, 128].

Sharding: 1D row partition of A_norm (2048 rows/core). Each core holds
A[rows_c].T (host-transposed so the node-contraction dim lands on SBUF
partitions), computes its row block of each layer, and the hidden state
is exchanged between layers with chunked on-device AllGathers.

Device layout is transpose-free:
  - aggregate:  psum[d, i] += X_tile[j, d].T @ A_T_tile[j, i]
                (lhsT = stationary node-major X/H tile, rhs = A^T slice)
  - linear:     psum[i, e]  = M^T_tile[d, i].T @ W[d, e]   (node-major out)
  - relu fused into the PSUM->SBUF eviction on the scalar engine.

The aggregation runs CHUNK-MAJOR (one 512-wide output chunk at a time,
full contraction each): chunk k's hidden tiles finish at ~(k+1)/4 of the
layer, so AllGather k overlaps the remaining chunks' compute — only the
last AllGather is exposed at the layer boundary. The stationary H layout
in SBUF ([128, 512] pieces) is exactly what the chunked AllGathers
produce, so no transposes are needed anywhere.

PRECISION modes:
  - "fp32":   exact fp32 matmuls (4 cyc/row on the PE).
  - "split3": A and X/H split into bf16 hi+lo; aggregate computed as
              Ah@Xh + Al@Xh + Ah@Xl (3 bf16 passes, ~2.5e-6 rel err —
              fp32-class).
  - "bf16":   plain bf16 aggregate (1 cyc/row, half the DMA bytes,
              ~1.1e-3 rel err).
"""

import sys
from contextlib import ExitStack

if "/opt/trn_rl_repo" not in sys.path:
    sys.path.insert(0, "/opt/trn_rl_repo")

import numpy as np

N_NODES = 16384
D = 128
NCORES = 8
ROWS = N_NODES // NCORES  # 2048

PRECISION = "bf16"  # "fp32" | "split3" | "bf16"


def _geom(n_nodes=N_NODES, ncores=NCORES, precision=PRECISION):
    esz = 4 if precision == "fp32" else 2
    nsplit = 2 if precision == "split3" else 1  # hi/lo operand copies
    rows = n_nodes // ncores
    jt = n_nodes // 128          # total j-tiles (contraction tiles)
    jt_per_rank = jt // ncores   # j-tiles covered by one rank's nodes
    ic = min(512, rows)          # i-chunk width (one PSUM bank, fp32 out)
    nch = rows // ic             # i-chunks per core
    # j-tiles per A DMA: ~2 MiB per transfer; deep buffer pool so the
    # A-stream prefetch (~16 MiB) covers the inter-layer AllGather window
    target = 2 * 1024 * 1024
    jg = max(1, target // (128 * rows * esz))
    jg = min(jg, jt)
    while jt % jg:
        jg -= 1
    a_bufs = {"bf16": 8, "fp32": 6, "split3": 3}[precision]
    return dict(
        esz=esz, nsplit=nsplit, rows=rows, jt=jt, jt_per_rank=jt_per_rank,
        ic=ic, nch=nch, jg=jg, ndma=jt // jg, a_bufs=a_bufs,
    )


def build_gcn(n_nodes=N_NODES, d=D, ncores=NCORES, precision=PRECISION):
    """Build the SPMD Bass program (one program, runs on all cores)."""
    import concourse.bass as bass  # noqa: F401
    import concourse.tile as tile
    from concourse import bacc, mybir

    F32 = mybir.dt.float32
    BF16 = mybir.dt.bfloat16
    agg_dt = F32 if precision == "fp32" else BF16

    g_ = _geom(n_nodes, ncores, precision)
    nsplit, rows, jt = g_["nsplit"], g_["rows"], g_["jt"]
    jt_per_rank, ic, nch = g_["jt_per_rank"], g_["ic"], g_["nch"]
    jg, ndma, a_bufs = g_["jg"], g_["ndma"], g_["a_bufs"]
    lt = ic // 128               # linear i-tiles (and h tiles) per chunk

    nc = bacc.Bacc("TRN2", target_bir_lowering=False, num_devices=ncores)

    # A^T shards, host pre-tiled: DMA group g is the contiguous block
    # a_in[g*128 : (g+1)*128, :], covering j-tiles [g*jg, (g+1)*jg) x
    # all output columns, with a_in[g*128+p, t*rows+i] = A^T[(g*jg+t)*128+p, i]
    a_in = [
        nc.dram_tensor(
            f"a{s}", [ndma * 128, jg * rows], agg_dt, kind="ExternalInput"
        )
        for s in range(nsplit)
    ]
    # x_t: X pre-tiled on host into the AllGather layout:
    # x_t[r*128 + p, tl*128 + dd] = X[(r*jt_per_rank + tl)*128 + p, dd]
    x_in = [
        nc.dram_tensor(f"x{s}", [ncores * 128, rows], agg_dt, kind="ExternalInput")
        for s in range(nsplit)
    ]
    w0 = nc.dram_tensor("w0", [d, d], F32, kind="ExternalInput")
    w1 = nc.dram_tensor("w1", [d, d], F32, kind="ExternalInput")
    h_out = nc.dram_tensor("h_out", [rows, d], F32, kind="ExternalOutput")

    relu = mybir.ActivationFunctionType.Relu

    with tile.TileContext(nc) as tc, ExitStack() as ctx:
        sb1 = ctx.enter_context(tc.tile_pool(name="sb1", bufs=1))
        stat_pool = ctx.enter_context(
            tc.tile_pool(name="stat", bufs=ncores * nsplit)
        )
        a_pool = ctx.enter_context(tc.tile_pool(name="a", bufs=a_bufs))
        m_pool = ctx.enter_context(tc.tile_pool(name="m", bufs=2))
        h_pool = ctx.enter_context(tc.tile_pool(name="h", bufs=4))
        split_pool = ctx.enter_context(tc.tile_pool(name="spl", bufs=4))
        agg_pool = ctx.enter_context(tc.tile_pool(name="agg", bufs=4, space="PSUM"))
        lin_pool = ctx.enter_context(tc.tile_pool(name="lin", bufs=2, space="PSUM"))
        dram = ctx.enter_context(tc.tile_pool(name="dram", bufs=1, space="DRAM"))

        w0_sb = sb1.tile([d, d], F32)
        nc.scalar.dma_start(out=w0_sb[:], in_=w0[:])
        w1_sb = sb1.tile([d, d], F32)
        nc.scalar.dma_start(out=w1_sb[:], in_=w1[:])

        def load_stat_chunks(srcs, lname):
            """srcs: per split s: [ncores*128, rows] DRAM view.
            Returns stat[s][r] = [128, rows] SBUF tile."""
            out = []
            for s in range(nsplit):
                chunks = []
                for r in range(ncores):
                    sc = stat_pool.tile(
                        [128, rows], agg_dt, name=f"{lname}{s}_{r}", tag="sc"
                    )
                    nc.gpsimd.dma_start(
                        out=sc[:], in_=srcs[s][r * 128 : (r + 1) * 128, :]
                    )
                    chunks.append(sc)
                out.append(chunks)
            return out

        def layer(stat, w_sb, write_out, layer_done):
            # stat[s][r]: stationary chunks; j-tile j lives in chunk
            # r=j//jt_per_rank at cols (j%jt_per_rank)*128
            passes = [(0, 0)] if nsplit == 1 else [(0, 0), (1, 0), (0, 1)]
            agg = [
                agg_pool.tile([128, ic], F32, name=f"ps{c}", tag="ps")
                for c in range(nch)
            ]
            for g in range(ndma):
                ats = []
                for s in range(nsplit):
                    at = a_pool.tile(
                        [128, jg * rows], agg_dt, name=f"at{s}", tag=f"at{s}"
                    )
                    eng = nc.sync if (g + s) % 2 == 0 else nc.scalar
                    eng.dma_start(
                        out=at[:], in_=a_in[s][g * 128 : (g + 1) * 128, :]
                    )
                    ats.append(at)
                for t in range(jg):
                    j = g * jg + t
                    jr = j % jt_per_rank
                    for pi, (ls, rs) in enumerate(passes):
                        lhs = stat[ls][j // jt_per_rank][
                            :, jr * 128 : (jr + 1) * 128
                        ]
                        for c in range(nch):
                            nc.tensor.matmul(
                                agg[c][:],
                                lhsT=lhs,
                                rhs=ats[rs][
                                    :, t * rows + c * ic : t * rows + (c + 1) * ic
                                ],
                                start=(j == 0 and pi == 0),
                                stop=(j == jt - 1 and pi == len(passes) - 1),
                            )
            # linear + relu, node-major output tiles
            for c in range(nch):
                mt = m_pool.tile([128, ic], F32, name="mt", tag="mt")
                nc.vector.tensor_copy(out=mt[:], in_=agg[c][:])
                for it in range(lt):
                    lp = lin_pool.tile([128, d], F32, name="lp", tag="lp")
                    nc.tensor.matmul(
                        lp[:],
                        lhsT=mt[:, it * 128 : (it + 1) * 128],
                        rhs=w_sb[:],
                        start=True,
                        stop=True,
                    )
                    ht = h_pool.tile([128, d], F32, name="ht", tag="ht")
                    nc.scalar.activation(ht[:], lp[:], relu)
                    write_out(c, it, ht)
            layer_done()

        # ---- layer 0 ----
        stat0 = load_stat_chunks([x[:] for x in x_in], "sx")
        # packed hidden-state bounce ([hi | lo] along free dim when split)
        h_tb = dram.tile([128, nsplit * rows], agg_dt, name="h_tb")
        h_ag = dram.tile(
            [ncores * 128, nsplit * rows], agg_dt, addr_space="Shared", name="h_ag"
        )

        def write_l0(c, it, ht):
            tl = c * lt + it
            if precision == "fp32":
                nc.scalar.dma_start(
                    out=h_tb[:, tl * 128 : (tl + 1) * 128], in_=ht[:]
                )
                return
            hh = split_pool.tile([128, d], BF16, name="hh", tag="hh")
            nc.vector.tensor_copy(out=hh[:], in_=ht[:])
            nc.scalar.dma_start(out=h_tb[:, tl * 128 : (tl + 1) * 128], in_=hh[:])
            if nsplit == 2:
                hh32 = split_pool.tile([128, d], F32, name="hh32", tag="hh32")
                nc.vector.tensor_copy(out=hh32[:], in_=hh[:])
                hl = split_pool.tile([128, d], BF16, name="hl", tag="hl")
                nc.vector.tensor_sub(out=hl[:], in0=ht[:], in1=hh32[:])
                nc.scalar.dma_start(
                    out=h_tb[:, rows + tl * 128 : rows + (tl + 1) * 128], in_=hl[:]
                )

        def ag_l0():
            import concourse.mybir as _mb

            nc.gpsimd.collective_compute(
                "AllGather",
                _mb.AluOpType.bypass,
                replica_groups=[list(range(ncores))],
                ins=[h_tb[:]],
                outs=[h_ag[:]],
            )

        layer(stat0, w0_sb, write_l0, ag_l0)

        # ---- layer 1 ----
        stat1 = load_stat_chunks(
            [h_ag[:, s * rows : (s + 1) * rows] for s in range(nsplit)], "sh"
        )

        def write_l1(c, it, ht):
            nc.scalar.dma_start(
                out=h_out[c * ic + it * 128 : c * ic + (it + 1) * 128, :], in_=ht[:]
            )

        layer(stat1, w1_sb, write_l1, lambda: None)

    nc.finalize()
    return nc


def _tile_stat(X, ncores, jt_per_rank):
    rows = jt_per_rank * 128
    return np.ascontiguousarray(
        X.reshape(ncores, jt_per_rank, 128, D).transpose(0, 2, 1, 3)
        .reshape(ncores * 128, rows)
    )


def shard_inputs(A_norm, X, n_nodes=N_NODES, ncores=NCORES, precision=PRECISION):
    """Host-side shard prep. Returns per-core input maps."""
    import ml_dtypes

    bf16 = ml_dtypes.bfloat16
    g_ = _geom(n_nodes, ncores, precision)
    rows, jt_per_rank = g_["rows"], g_["jt_per_rank"]
    jg, ndma = g_["jg"], g_["ndma"]

    def tile_a(a_tc):
        # [n_nodes, rows] -> [ndma*128, jg*rows] so DMA group g is the
        # contiguous block a_pre[g*128:(g+1)*128, :] with
        # a_pre[g*128+p, t*rows+i] = a_tc[(g*jg+t)*128+p, i]
        return np.ascontiguousarray(
            a_tc.reshape(ndma, jg, 128, rows).swapaxes(1, 2)
            .reshape(ndma * 128, jg * rows)
        )

    x_t = _tile_stat(X, ncores, jt_per_rank)
    if precision == "fp32":
        xs = [x_t]
    else:
        x_hi = x_t.astype(bf16)
        xs = [x_hi]
        if precision == "split3":
            xs.append((x_t - x_hi.astype(np.float32)).astype(bf16))

    in_maps = []
    for c in range(ncores):
        a_tc = np.ascontiguousarray(A_norm[c * rows : (c + 1) * rows, :].T)
        m = {}
        if precision == "fp32":
            m["a0"] = tile_a(a_tc)
        else:
            a_hi = a_tc.astype(bf16)
            m["a0"] = tile_a(a_hi)
            if precision == "split3":
                m["a1"] = tile_a((a_tc - a_hi.astype(np.float32)).astype(bf16))
        for s, x in enumerate(xs):
            m[f"x{s}"] = x
        in_maps.append(m)
    return in_maps


_CACHED = {}


def kernel(A_norm, X, W0, W1):
    A_norm = np.ascontiguousarray(A_norm, dtype=np.float32)
    X = np.ascontiguousarray(X, dtype=np.float32)
    W0 = np.ascontiguousarray(W0, dtype=np.float32)
    W1 = np.ascontiguousarray(W1, dtype=np.float32)

    from concourse.bass_utils import run_bass_kernel_spmd

    if PRECISION not in _CACHED:
        _CACHED[PRECISION] = build_gcn(precision=PRECISION)
    nc = _CACHED[PRECISION]

    in_maps = shard_inputs(A_norm, X, precision=PRECISION)
    for m in in_maps:
        m["w0"] = W0
        m["w1"] = W1

    res = run_bass_kernel_spmd(nc, in_maps, core_ids=list(range(NCORES)))
    return np.concatenate([res.results[c]["h_out"] for c in range(NCORES)], axis=0)



# revision 2
# speedup vs baseline: 1.4728x; 1.4728x over previous
"""2-layer dense GCN on 8 Trainium2 NeuronCores — fp8-A streaming version.

Reference computation (all fp32):
    H0 = relu((A_norm @ X) @ W0)
    H1 = relu((A_norm @ H0) @ W1)
A_norm: [16384, 16384], X: [16384, 128], W0/W1: [128, 128].

Sharding: 1D row partition of A_norm (2048 rows/core). Each core streams
its A^T shard quantized to fp8e4 (scaled by 2^13; W pre-divided by the
scale on host), with the stationary X/H operand held in bf16 SBUF tiles.
The tensor engine runs mixed-dtype matmuls (bf16 lhsT x fp8 rhs) at bf16
speed, so halving the A bytes moves the kernel from DMA-bound to the
compute/DMA ridge.

Schedule: CHUNK-MAJOR aggregation — each 512-row output chunk runs its
full 16384-deep contraction before the next chunk starts, so chunk c
finishes at (c+1)/4 of the layer. Its linear+relu output is AllGathered
immediately (4 chunked collectives), overlapping the remaining chunks'
compute. Layer 1 consumes the gathered H0 in PIECE-MAJOR j-order (the
host pre-permutes the A layout to match), so its contraction can begin
as soon as the first AllGather piece lands; the same permuted A layout
is reused by layer 0 (order is irrelevant there), so both layers stream
one DRAM tensor.

Numerics (host-simulated end-to-end): rel err ~4.3e-3 vs fp32 reference
(A fp8e4 + bf16 stationaries/aggregates/weights).
"""

import sys
from contextlib import ExitStack

if "/opt/trn_rl_repo" not in sys.path:
    sys.path.insert(0, "/opt/trn_rl_repo")

import numpy as np

N_NODES = 16384
D = 128
NCORES = 8
ROWS = N_NODES // NCORES     # 2048 output rows per core
NCH = 4                      # output chunks per core (PSUM-bank sized)
IC = ROWS // NCH             # 512 = chunk width (one fp32 PSUM bank)
JT = N_NODES // 128          # 128 contraction j-tiles
GJ = 8                       # j-tiles per A DMA group (512 KB transfers)
NG = JT // GJ                # 16 DMA groups per chunk
A_BUFS = 16                  # A-stream SBUF ring (8 MiB prefetch depth)
SCALE = 2.0 ** 13            # A quantization scale; W is divided by it

PRECISION = "fp8"


def _jorder():
    """Stream order of global j-tiles: piece-major (p, r, tl) so layer 1
    can start on AllGather piece 0 before later pieces arrive."""
    order = []
    for k in range(JT):
        p, r, tl = k // 32, (k % 32) // 4, k % 4
        order.append((p, r, tl))
    return order


def build_gcn():
    import concourse.bass as bass  # noqa: F401
    import concourse.tile as tile
    from concourse import bacc, mybir

    F32 = mybir.dt.float32
    BF16 = mybir.dt.bfloat16
    F8 = mybir.dt.float8e4

    nc = bacc.Bacc("TRN2", target_bir_lowering=False, num_devices=NCORES)

    # host-pretiled A^T shard, fp8, piece-major j order:
    # a_in[(cc*NG+g)*128 + p, t*IC + i] = SCALE * A^T[jorder[g*GJ+t]*128 + p,
    #                                                cc*IC + i]
    a_in = nc.dram_tensor("a", [NCH * NG * 128, GJ * IC], F8, kind="ExternalInput")
    # x_in[r*128 + p, jl*128 + dd] = X[(r*16 + jl)*128 + p, dd]  (bf16)
    x_in = nc.dram_tensor("x0", [NCORES * 128, ROWS], BF16, kind="ExternalInput")
    w0 = nc.dram_tensor("w0", [D, D], BF16, kind="ExternalInput")  # W0 / SCALE
    w1 = nc.dram_tensor("w1", [D, D], BF16, kind="ExternalInput")  # W1 / SCALE
    h_out = nc.dram_tensor("h_out", [ROWS, D], F32, kind="ExternalOutput")

    relu = mybir.ActivationFunctionType.Relu
    order = _jorder()

    with tile.TileContext(nc) as tc, ExitStack() as ctx:
        sb1 = ctx.enter_context(tc.tile_pool(name="sb1", bufs=1))
        sx_pool = ctx.enter_context(tc.tile_pool(name="sx", bufs=1))
        sh_pool = ctx.enter_context(tc.tile_pool(name="sh", bufs=1))
        a_pool = ctx.enter_context(tc.tile_pool(name="a", bufs=A_BUFS))
        m_pool = ctx.enter_context(tc.tile_pool(name="m", bufs=2))
        h_pool = ctx.enter_context(tc.tile_pool(name="h", bufs=4))
        agg_pool = ctx.enter_context(tc.tile_pool(name="agg", bufs=2, space="PSUM"))
        lin_pool = ctx.enter_context(tc.tile_pool(name="lin", bufs=2, space="PSUM"))
        dram = ctx.enter_context(tc.tile_pool(name="dram", bufs=1, space="DRAM"))

        w0_sb = sb1.tile([D, D], BF16, name="w0_sb", tag="w0")
        nc.scalar.dma_start(out=w0_sb[:], in_=w0[:])
        w1_sb = sb1.tile([D, D], BF16, name="w1_sb", tag="w1")
        nc.scalar.dma_start(out=w1_sb[:], in_=w1[:])

        # stationary X chunks (bf16), one per source rank
        stat_x = []
        for r in range(NCORES):
            sx = sx_pool.tile([128, ROWS], BF16, name=f"sx{r}", tag=f"sx{r}")
            nc.gpsimd.dma_start(out=sx[:], in_=x_in[r * 128 : (r + 1) * 128, :])
            stat_x.append(sx)

        # stationary H0 tiles, one per (rank, piece), filled as AGs land
        sh = {}
        for p in range(NCH):
            for r in range(NCORES):
                sht = sh_pool.tile([128, IC], BF16, name=f"sh{r}_{p}", tag=f"sh{r}_{p}")
                sh[(r, p)] = sht

        h_tb = [
            dram.tile([128, IC], BF16, name=f"htb{p}", tag=f"tb{p}")
            for p in range(NCH)
        ]
        h_ag = [
            dram.tile(
                [NCORES * 128, IC], BF16, addr_space="Shared",
                name=f"hag{p}", tag=f"ag{p}",
            )
            for p in range(NCH)
        ]

        dma_ctr = [0]

        def stream_layer(lhsT_for, w_sb, emit_out, chunk_done):
            for cc in range(NCH):
                agg = agg_pool.tile([128, IC], F32, name="ps", tag="ps")
                for g in range(NG):
                    at = a_pool.tile([128, GJ * IC], F8, name="at", tag="at")
                    eng = nc.sync if dma_ctr[0] % 2 == 0 else nc.scalar
                    dma_ctr[0] += 1
                    row0 = (cc * NG + g) * 128
                    eng.dma_start(out=at[:], in_=a_in[row0 : row0 + 128, :])
                    for t in range(GJ):
                        k = g * GJ + t
                        nc.tensor.matmul(
                            agg[:],
                            lhsT=lhsT_for(k),
                            rhs=at[:, t * IC : (t + 1) * IC],
                            start=(k == 0),
                            stop=(k == JT - 1),
                        )
                mt = m_pool.tile([128, IC], BF16, name="mt", tag="mt")
                nc.vector.tensor_copy(out=mt[:], in_=agg[:])
                for it in range(IC // 128):
                    lp = lin_pool.tile([128, D], F32, name="lp", tag="lp")
                    nc.tensor.matmul(
                        lp[:],
                        lhsT=mt[:, it * 128 : (it + 1) * 128],
                        rhs=w_sb[:],
                        start=True,
                        stop=True,
                    )
                    emit_out(cc, it, lp)
                chunk_done(cc)

        # ---- layer 0 ----
        def lhsT0(k):
            p, r, tl = order[k]
            jl = p * 4 + tl
            return stat_x[r][:, jl * 128 : (jl + 1) * 128]

        def emit0(cc, it, lp):
            hh = h_pool.tile([128, D], BF16, name="hh", tag="hh")
            nc.scalar.activation(hh[:], lp[:], relu)
            nc.scalar.dma_start(out=h_tb[cc][:, it * 128 : (it + 1) * 128], in_=hh[:])

        def done0(cc):
            nc.gpsimd.collective_compute(
                "AllGather",
                mybir.AluOpType.bypass,
                replica_groups=[list(range(NCORES))],
                ins=[h_tb[cc][:]],
                outs=[h_ag[cc][:]],
            )
            for r in range(NCORES):
                nc.gpsimd.dma_start(
                    out=sh[(r, cc)][:], in_=h_ag[cc][r * 128 : (r + 1) * 128, :]
                )

        stream_layer(lhsT0, w0_sb, emit0, done0)

        # ---- layer 1 ----
        def lhsT1(k):
            p, r, tl = order[k]
            return sh[(r, p)][:, tl * 128 : (tl + 1) * 128]

        def emit1(cc, it, lp):
            ht = h_pool.tile([128, D], F32, name="ht", tag="ht")
            nc.scalar.activation(ht[:], lp[:], relu)
            nc.scalar.dma_start(
                out=h_out[cc * IC + it * 128 : cc * IC + (it + 1) * 128, :],
                in_=ht[:],
            )

        stream_layer(lhsT1, w1_sb, emit1, lambda cc: None)

    nc.finalize()
    return nc


def _tile_stat(X):
    return np.ascontiguousarray(
        X.reshape(NCORES, JT // NCORES, 128, D).transpose(0, 2, 1, 3)
        .reshape(NCORES * 128, ROWS)
    )


def shard_inputs(A_norm, X, W0, W1):
    """Host-side shard prep. Returns per-core input maps."""
    import ml_dtypes

    bf16 = ml_dtypes.bfloat16
    e4 = ml_dtypes.float8_e4m3

    x_t = _tile_stat(X).astype(bf16)
    w0b = (W0 / SCALE).astype(bf16)
    w1b = (W1 / SCALE).astype(bf16)
    jorder = np.array([p * 4 + r * 16 + tl for (p, r, tl) in _jorder()])

    in_maps = []
    for c in range(NCORES):
        a_tc = A_norm[c * ROWS : (c + 1) * ROWS, :].T  # [16384, 2048] view
        aq = (a_tc * np.float32(SCALE)).astype(e4)
        # [j_tile, p, cc, i] -> permute j by stream order -> [cc, g, p, t, i]
        aq4 = aq.reshape(JT, 128, NCH, IC)
        arr = (
            aq4[jorder]
            .reshape(NG, GJ, 128, NCH, IC)
            .transpose(3, 0, 2, 1, 4)
            .reshape(NCH * NG * 128, GJ * IC)
        )
        in_maps.append(
            {"a": np.ascontiguousarray(arr), "x0": x_t, "w0": w0b, "w1": w1b}
        )
    return in_maps


_CACHED = {}


def kernel(A_norm, X, W0, W1):
    A_norm = np.ascontiguousarray(A_norm, dtype=np.float32)
    X = np.ascontiguousarray(X, dtype=np.float32)
    W0 = np.ascontiguousarray(W0, dtype=np.float32)
    W1 = np.ascontiguousarray(W1, dtype=np.float32)

    from concourse.bass_utils import run_bass_kernel_spmd

    if PRECISION not in _CACHED:
        _CACHED[PRECISION] = build_gcn()
    nc = _CACHED[PRECISION]

    in_maps = shard_inputs(A_norm, X, W0, W1)
    res = run_bass_kernel_spmd(nc, in_maps, core_ids=list(range(NCORES)))
    return np.concatenate([res.results[c]["h_out"] for c in range(NCORES)], axis=0)


# revision 3
# speedup vs baseline: 1.5106x; 1.0256x over previous
"""2-layer dense GCN on 8 Trainium2 NeuronCores — fp8-A streaming version.

Reference computation (all fp32):
    H0 = relu((A_norm @ X) @ W0)
    H1 = relu((A_norm @ H0) @ W1)
A_norm: [16384, 16384], X: [16384, 128], W0/W1: [128, 128].

Sharding: 1D row partition of A_norm (2048 rows/core). Each core streams
its A^T shard quantized to fp8e4 (scaled by 2^13; W pre-divided by the
scale on host), with the stationary X/H operand held in bf16 SBUF tiles.
The tensor engine runs mixed-dtype matmuls (bf16 lhsT x fp8 rhs) at bf16
speed, so halving the A bytes moves the kernel from DMA-bound to the
compute/DMA ridge (PE ~131us/layer at the P0 2.0 GHz clock, A stream
~115us/layer).

Schedule: CHUNK-MAJOR aggregation — each output chunk runs its full
16384-deep contraction before the next chunk starts. Layer 0 uses
asymmetric chunks [512,512,512,256,256] whose linear+relu outputs are
AllGathered piece-by-piece (5 chunked collectives), overlapping the
remaining chunks' compute; the two small tail pieces finish their
gathers before layer 1 needs them. Layer 0's contraction runs in
rank-major j-order so each source rank's stationary X tile is needed
progressively (~8us apart), letting the X loads trickle in behind the
A stream. Layer 1 runs in piece-major j-order so its contraction can
begin as soon as AllGather piece 0 lands.

Numerics (host-simulated end-to-end): rel err ~4.3e-3 vs fp32 reference
(A fp8e4 + bf16 stationaries/aggregates/weights); measured 5.0e-3 on HW.
"""

import sys
from contextlib import ExitStack

if "/opt/trn_rl_repo" not in sys.path:
    sys.path.insert(0, "/opt/trn_rl_repo")

import numpy as np

N_NODES = 16384
D = 128
NCORES = 8
ROWS = N_NODES // NCORES     # 2048 output rows per core
JT = N_NODES // 128          # 128 contraction j-tiles
JPR = JT // NCORES           # 16 j-tiles per source rank
GCOLS = 4096                 # uniform A DMA group width (512 KB fp8)
A_BUFS = 16                  # A-stream SBUF ring (8 MiB prefetch depth)
SCALE = 2.0 ** 13            # A quantization scale; W is divided by it

# layer-0 output chunks == AllGather pieces (offset, width)
CHUNKS0 = [(0, 512), (512, 512), (1024, 512), (1536, 256), (1792, 256)]
CHUNKS1 = [(0, 512), (512, 512), (1024, 512), (1536, 512)]
NP0 = len(CHUNKS0)

PRECISION = "fp8"


def _order0():
    """Layer-0 stream order: rank-major (stationary X tiles needed
    progressively, so their loads hide behind the A stream)."""
    return [(k // JPR, k % JPR) for k in range(JT)]  # (r, jl)


def _order1():
    """Layer-1 stream order: piece-major over layer-0's output pieces,
    so aggregation starts as soon as AllGather piece 0 lands."""
    order = []
    for p, (off, w) in enumerate(CHUNKS0):
        for r in range(NCORES):
            for tl in range(w // 128):
                order.append((p, r, tl))
    return order


def _j_of(order, chunks0=CHUNKS0):
    """Global j-tile index for each stream slot."""
    js = []
    for item in order:
        if len(item) == 2:  # (r, jl)
            r, jl = item
            js.append(r * JPR + jl)
        else:  # (p, r, tl)
            p, r, tl = item
            off = chunks0[p][0]
            js.append(r * JPR + off // 128 + tl)
    return np.array(js)


def build_gcn():
    import concourse.bass as bass  # noqa: F401
    import concourse.tile as tile
    from concourse import bacc, mybir

    F32 = mybir.dt.float32
    BF16 = mybir.dt.bfloat16
    F8 = mybir.dt.float8e4

    nc = bacc.Bacc("TRN2", target_bir_lowering=False, num_devices=NCORES)

    ngroups0 = sum(JT // (GCOLS // w) for _, w in CHUNKS0)  # 64
    ngroups1 = sum(JT // (GCOLS // w) for _, w in CHUNKS1)  # 64
    # host-pretiled A^T shard, fp8: per chunk, DMA groups of gj j-tiles
    # laid out as [128, gj*w] with the chunk's column window, in the
    # layer's stream order.
    a0_in = nc.dram_tensor("a0", [ngroups0 * 128, GCOLS], F8, kind="ExternalInput")
    a1_in = nc.dram_tensor("a1", [ngroups1 * 128, GCOLS], F8, kind="ExternalInput")
    # x_in[r*128 + p, jl*128 + dd] = X[(r*16 + jl)*128 + p, dd]  (bf16)
    x_in = nc.dram_tensor("x0", [NCORES * 128, ROWS], BF16, kind="ExternalInput")
    w0 = nc.dram_tensor("w0", [D, D], BF16, kind="ExternalInput")  # W0 / SCALE
    w1 = nc.dram_tensor("w1", [D, D], BF16, kind="ExternalInput")  # W1 / SCALE
    h_out = nc.dram_tensor("h_out", [ROWS, D], F32, kind="ExternalOutput")

    relu = mybir.ActivationFunctionType.Relu
    order0, order1 = _order0(), _order1()

    with tile.TileContext(nc) as tc, ExitStack() as ctx:
        sb1 = ctx.enter_context(tc.tile_pool(name="sb1", bufs=1))
        sx_pool = ctx.enter_context(tc.tile_pool(name="sx", bufs=1))
        sh_pool = ctx.enter_context(tc.tile_pool(name="sh", bufs=1))
        a_pool = ctx.enter_context(tc.tile_pool(name="a", bufs=A_BUFS))
        m_pool = ctx.enter_context(tc.tile_pool(name="m", bufs=2))
        h_pool = ctx.enter_context(tc.tile_pool(name="h", bufs=4))
        agg_pool = ctx.enter_context(tc.tile_pool(name="agg", bufs=2, space="PSUM"))
        lin_pool = ctx.enter_context(tc.tile_pool(name="lin", bufs=2, space="PSUM"))
        dram = ctx.enter_context(tc.tile_pool(name="dram", bufs=1, space="DRAM"))

        dma_ctr = [0]

        def a_eng():
            eng = nc.sync if dma_ctr[0] % 2 == 0 else nc.scalar
            dma_ctr[0] += 1
            return eng

        w0_sb = sb1.tile([D, D], BF16, name="w0_sb", tag="w0")
        nc.scalar.dma_start(out=w0_sb[:], in_=w0[:])
        w1_sb = sb1.tile([D, D], BF16, name="w1_sb", tag="w1")
        nc.scalar.dma_start(out=w1_sb[:], in_=w1[:])

        # stationary X chunks (bf16), one per source rank, loaded in two
        # halves in rank order (rank r first needed ~8us * r into layer 0)
        stat_x = []
        for r in range(NCORES):
            sx = sx_pool.tile([128, ROWS], BF16, name=f"sx{r}", tag=f"sx{r}")
            stat_x.append(sx)
        for r in range(NCORES):
            for hh in range(2):
                c0, c1 = hh * (ROWS // 2), (hh + 1) * (ROWS // 2)
                nc.gpsimd.dma_start(
                    out=stat_x[r][:, c0:c1],
                    in_=x_in[r * 128 : (r + 1) * 128, c0:c1],
                )

        # stationary H0 tiles, one per (rank, piece), filled as AGs land
        sh = {}
        for p, (off, w) in enumerate(CHUNKS0):
            for r in range(NCORES):
                sh[(r, p)] = sh_pool.tile(
                    [128, w], BF16, name=f"sh{r}_{p}", tag=f"sh{r}_{p}"
                )

        h_tb = [
            dram.tile([128, w], BF16, name=f"htb{p}", tag=f"tb{p}")
            for p, (off, w) in enumerate(CHUNKS0)
        ]
        h_ag = [
            dram.tile(
                [NCORES * 128, w], BF16, addr_space="Shared",
                name=f"hag{p}", tag=f"ag{p}",
            )
            for p, (off, w) in enumerate(CHUNKS0)
        ]

        def stream_layer(chunks, a_in, lhsT_for, w_sb, emit_out, chunk_done):
            row = 0
            for c, (off, w) in enumerate(chunks):
                gj = GCOLS // w
                agg = agg_pool.tile([128, w], F32, name="ps", tag="ps")
                for g in range(JT // gj):
                    at = a_pool.tile([128, GCOLS], F8, name="at", tag="at")
                    a_eng().dma_start(out=at[:], in_=a_in[row : row + 128, :])
                    row += 128
                    for t in range(gj):
                        k = g * gj + t
                        nc.tensor.matmul(
                            agg[:],
                            lhsT=lhsT_for(k),
                            rhs=at[:, t * w : (t + 1) * w],
                            start=(k == 0),
                            stop=(k == JT - 1),
                        )
                mt = m_pool.tile([128, w], BF16, name="mt", tag="mt")
                nc.vector.tensor_copy(out=mt[:], in_=agg[:])
                for it in range(w // 128):
                    lp = lin_pool.tile([128, D], F32, name="lp", tag="lp")
                    nc.tensor.matmul(
                        lp[:],
                        lhsT=mt[:, it * 128 : (it + 1) * 128],
                        rhs=w_sb[:],
                        start=True,
                        stop=True,
                    )
                    emit_out(c, it, lp)
                chunk_done(c)

        # ---- layer 0 ----
        def lhsT0(k):
            r, jl = order0[k]
            return stat_x[r][:, jl * 128 : (jl + 1) * 128]

        def emit0(cc, it, lp):
            hh = h_pool.tile([128, D], BF16, name="hh", tag="hh")
            nc.scalar.activation(hh[:], lp[:], relu)
            nc.scalar.dma_start(out=h_tb[cc][:, it * 128 : (it + 1) * 128], in_=hh[:])

        def done0(cc):
            nc.gpsimd.collective_compute(
                "AllGather",
                mybir.AluOpType.bypass,
                replica_groups=[list(range(NCORES))],
                ins=[h_tb[cc][:]],
                outs=[h_ag[cc][:]],
            )
            for r in range(NCORES):
                nc.gpsimd.dma_start(
                    out=sh[(r, cc)][:], in_=h_ag[cc][r * 128 : (r + 1) * 128, :]
                )

        stream_layer(CHUNKS0, a0_in, lhsT0, w0_sb, emit0, done0)

        # ---- layer 1 ----
        def lhsT1(k):
            p, r, tl = order1[k]
            return sh[(r, p)][:, tl * 128 : (tl + 1) * 128]

        def emit1(cc, it, lp):
            ht = h_pool.tile([128, D], F32, name="ht", tag="ht")
            nc.scalar.activation(ht[:], lp[:], relu)
            nc.scalar.dma_start(
                out=h_out[cc * 512 + it * 128 : cc * 512 + (it + 1) * 128, :],
                in_=ht[:],
            )

        stream_layer(CHUNKS1, a1_in, lhsT1, w1_sb, emit1, lambda cc: None)

    nc.finalize()
    return nc


def _tile_stat(X):
    return np.ascontiguousarray(
        X.reshape(NCORES, JPR, 128, D).transpose(0, 2, 1, 3)
        .reshape(NCORES * 128, ROWS)
    )


def _tile_a(aq4, chunks, jorder):
    """aq4: [JT, 128, 2048] quantized A^T tiles. Returns the DMA-group
    layout [ngroups*128, GCOLS] for the given chunking and j stream order."""
    parts = []
    for off, w in chunks:
        gj = GCOLS // w
        arr = aq4[jorder][:, :, off : off + w]          # [JT, 128, w]
        arr = arr.reshape(JT // gj, gj, 128, w).transpose(0, 2, 1, 3)
        parts.append(arr.reshape((JT // gj) * 128, gj * w))
    return np.ascontiguousarray(np.vstack(parts))


def shard_inputs(A_norm, X, W0, W1):
    """Host-side shard prep. Returns per-core input maps."""
    import ml_dtypes

    bf16 = ml_dtypes.bfloat16
    e4 = ml_dtypes.float8_e4m3

    x_t = _tile_stat(X).astype(bf16)
    w0b = (W0 / SCALE).astype(bf16)
    w1b = (W1 / SCALE).astype(bf16)
    j0 = _j_of(_order0())
    j1 = _j_of(_order1())

    in_maps = []
    for c in range(NCORES):
        a_tc = A_norm[c * ROWS : (c + 1) * ROWS, :].T  # [16384, 2048] view
        aq4 = (a_tc * np.float32(SCALE)).astype(e4).reshape(JT, 128, ROWS)
        in_maps.append(
            {
                "a0": _tile_a(aq4, CHUNKS0, j0),
                "a1": _tile_a(aq4, CHUNKS1, j1),
                "x0": x_t,
                "w0": w0b,
                "w1": w1b,
            }
        )
    return in_maps


_CACHED = {}


def kernel(A_norm, X, W0, W1):
    A_norm = np.ascontiguousarray(A_norm, dtype=np.float32)
    X = np.ascontiguousarray(X, dtype=np.float32)
    W0 = np.ascontiguousarray(W0, dtype=np.float32)
    W1 = np.ascontiguousarray(W1, dtype=np.float32)

    from concourse.bass_utils import run_bass_kernel_spmd

    if PRECISION not in _CACHED:
        _CACHED[PRECISION] = build_gcn()
    nc = _CACHED[PRECISION]

    in_maps = shard_inputs(A_norm, X, W0, W1)
    res = run_bass_kernel_spmd(nc, in_maps, core_ids=list(range(NCORES)))
    return np.concatenate([res.results[c]["h_out"] for c in range(NCORES)], axis=0)


# revision 6
# speedup vs baseline: 1.6971x; 1.1235x over previous
"""2-layer dense GCN on 8 Trainium2 NeuronCores — fp8-A streaming version.

Reference computation (all fp32):
    H0 = relu((A_norm @ X) @ W0)
    H1 = relu((A_norm @ H0) @ W1)
A_norm: [16384, 16384], X: [16384, 128], W0/W1: [128, 128].

Sharding: 1D row partition of A_norm (2048 rows/core). Each core streams
its A^T shard quantized to fp8e4 (scaled by 2^13; W pre-divided by the
scale on host), with the stationary X/H operand held in bf16 SBUF tiles.
The tensor engine runs mixed-dtype matmuls (bf16 lhsT x fp8 rhs) at bf16
speed, so halving the A bytes moves the kernel from DMA-bound to the
compute/DMA ridge (PE ~131us/layer at the P0 2.0 GHz clock, A stream
~115us/layer).

Schedule:
- Layer 0 runs CHUNK-MAJOR (4 x 512-wide output chunks, each completing
  its full 16384-deep contraction in rank-major j-order so the
  stationary X loads trickle in behind the A stream). Each finished
  chunk's linear+relu output is AllGathered immediately; the collective
  chain (which cannot start before the ~75us launch-skew barrier)
  overlaps layer-0/1 compute.
- Layer 1 runs PIECE-OUTER: for each AllGather piece p, it feeds piece
  p's j-tiles into all 4 output chunks' PSUM accumulators (4 banks open
  simultaneously). Demand for piece p is at L0_end + p*33us while the
  serialized AG chain supplies it at ~87 + p*25us, so the collectives
  are fully hidden.
- AG triggers are issued on gpsimd BEFORE the previous piece's
  stationary-H loads so triggers never queue behind DMA issues that
  block on collective completion.

Numerics (host-simulated end-to-end): rel err ~4.3e-3 vs fp32 reference
(A fp8e4 + bf16 stationaries/aggregates/weights); measured 5.0e-3 on HW.
"""

import sys
from contextlib import ExitStack

if "/opt/trn_rl_repo" not in sys.path:
    sys.path.insert(0, "/opt/trn_rl_repo")

import numpy as np

N_NODES = 16384
D = 128
NCORES = 8
ROWS = N_NODES // NCORES     # 2048 output rows per core
JT = N_NODES // 128          # 128 contraction j-tiles
JPR = JT // NCORES           # 16 j-tiles per source rank
NCH = 4                      # output chunks / AG pieces per core
IC = ROWS // NCH             # 512 chunk width (one fp32 PSUM bank)
PJT = IC // 128              # 4 j-tiles per rank per piece
A_BUFS = 16                  # A-stream SBUF ring (8 MiB prefetch depth)
SCALE = 2.0 ** 13            # A quantization scale; W is divided by it

# layer-0 chunk-0 DMA group sizes (j-tiles): small first transfers so the
# first matmul can start early; steady state 8-tile (512 KB) groups.
GJ0 = [2, 2, 4] + [8] * 15
GJS = [8] * 16               # all other layer-0 chunks
GJ1 = [8] * 4                # per (piece, chunk) block in layer 1

PRECISION = "fp8"


def _l0_stream():
    """Layer-0 A stream: per chunk, rank-major j order, grouped."""
    stream = []  # (chunk, [j-tiles], width)
    for c in range(NCH):
        gjs = GJ0 if c == 0 else GJS
        k = 0
        for gj in gjs:
            js = [k + t for t in range(gj)]  # j-tile index == stream pos
            stream.append((c, js, IC))
            k += gj
    return stream


def _l1_stream():
    """Layer-1 A stream: piece-outer, then output chunk, then j-tiles."""
    stream = []
    for p in range(NCH):
        pj = [r * JPR + p * PJT + tl for r in range(NCORES) for tl in range(PJT)]
        for c in range(NCH):
            for g in range(len(GJ1)):
                stream.append((p, c, pj[g * 8 : (g + 1) * 8]))
    return stream


def build_gcn():
    import concourse.bass as bass  # noqa: F401
    import concourse.tile as tile
    from concourse import bacc, mybir

    F32 = mybir.dt.float32
    BF16 = mybir.dt.bfloat16
    F8 = mybir.dt.float8e4

    nc = bacc.Bacc("TRN2", target_bir_lowering=False, num_devices=NCORES)

    n0 = sum(len(js) for _, js, _ in _l0_stream())   # 512 j-tile slots
    rows0 = len(_l0_stream()) * 128
    rows1 = len(_l1_stream()) * 128
    a0_in = nc.dram_tensor("a0", [rows0, 8 * IC], F8, kind="ExternalInput")
    a1_in = nc.dram_tensor("a1", [rows1, 8 * IC], F8, kind="ExternalInput")
    x_in = nc.dram_tensor("x0", [NCORES * 128, ROWS], BF16, kind="ExternalInput")
    w0 = nc.dram_tensor("w0", [D, D], BF16, kind="ExternalInput")  # W0 / SCALE
    w1 = nc.dram_tensor("w1", [D, D], BF16, kind="ExternalInput")  # W1 / SCALE
    h_out = nc.dram_tensor("h_out", [ROWS, D], F32, kind="ExternalOutput")
    assert n0 == JT * NCH

    relu = mybir.ActivationFunctionType.Relu

    with tile.TileContext(nc) as tc, ExitStack() as ctx:
        sb1 = ctx.enter_context(tc.tile_pool(name="sb1", bufs=1))
        sx_pool = ctx.enter_context(tc.tile_pool(name="sx", bufs=1))
        sh_pool = ctx.enter_context(tc.tile_pool(name="sh", bufs=1))
        a_pool = ctx.enter_context(tc.tile_pool(name="a", bufs=A_BUFS))
        m_pool = ctx.enter_context(tc.tile_pool(name="m", bufs=2))
        h_pool = ctx.enter_context(tc.tile_pool(name="h", bufs=4))
        agg_pool = ctx.enter_context(tc.tile_pool(name="agg", bufs=2, space="PSUM"))
        ag1_pool = ctx.enter_context(tc.tile_pool(name="agg1", bufs=1, space="PSUM"))
        lin_pool = ctx.enter_context(tc.tile_pool(name="lin", bufs=2, space="PSUM"))
        dram = ctx.enter_context(tc.tile_pool(name="dram", bufs=1, space="DRAM"))

        dma_ctr = [0]

        def a_eng():
            eng = nc.sync if dma_ctr[0] % 2 == 0 else nc.scalar
            dma_ctr[0] += 1
            return eng

        w0_sb = sb1.tile([D, D], BF16, name="w0_sb", tag="w0")
        nc.scalar.dma_start(out=w0_sb[:], in_=w0[:])
        w1_sb = sb1.tile([D, D], BF16, name="w1_sb", tag="w1")
        nc.scalar.dma_start(out=w1_sb[:], in_=w1[:])

        # stationary X chunks (bf16); rank r first needed ~8us*r into the
        # stream, so load rank-by-rank (first rank in two halves).
        stat_x = [
            sx_pool.tile([128, ROWS], BF16, name=f"sx{r}", tag=f"sx{r}")
            for r in range(NCORES)
        ]
        for r in range(NCORES):
            splits = [0, ROWS // 4, ROWS] if r == 0 else [0, ROWS]
            for c0, c1 in zip(splits[:-1], splits[1:]):
                nc.gpsimd.dma_start(
                    out=stat_x[r][:, c0:c1],
                    in_=x_in[r * 128 : (r + 1) * 128, c0:c1],
                )

        # stationary H0 tiles, one per (rank, piece), filled as AGs land
        sh = {
            (r, p): sh_pool.tile([128, IC], BF16, name=f"sh{r}_{p}", tag=f"sh{r}_{p}")
            for p in range(NCH)
            for r in range(NCORES)
        }

        h_tb = [
            dram.tile([128, IC], BF16, name=f"htb{p}", tag=f"tb{p}")
            for p in range(NCH)
        ]
        h_ag = [
            dram.tile(
                [NCORES * 128, IC], BF16, addr_space="Shared",
                name=f"hag{p}", tag=f"ag{p}",
            )
            for p in range(NCH)
        ]

        def load_stats(p):
            for r in range(NCORES):
                nc.gpsimd.dma_start(
                    out=sh[(r, p)][:], in_=h_ag[p][r * 128 : (r + 1) * 128, :]
                )

        def linear(agg, w_sb, emit_out, cc):
            mt = m_pool.tile([128, IC], BF16, name="mt", tag="mt")
            nc.vector.tensor_copy(out=mt[:], in_=agg[:])
            for it in range(IC // 128):
                lp = lin_pool.tile([128, D], F32, name="lp", tag="lp")
                nc.tensor.matmul(
                    lp[:],
                    lhsT=mt[:, it * 128 : (it + 1) * 128],
                    rhs=w_sb[:],
                    start=True,
                    stop=True,
                )
                emit_out(cc, it, lp)

        # ---- layer 0 ----
        def emit0(cc, it, lp):
            hh = h_pool.tile([128, D], BF16, name="hh", tag="hh")
            nc.scalar.activation(hh[:], lp[:], relu)
            nc.scalar.dma_start(out=h_tb[cc][:, it * 128 : (it + 1) * 128], in_=hh[:])

        row = 0
        cur = {"agg": None, "k": 0}
        for cc, js, w in _l0_stream():
            if cur["k"] == 0:
                cur["agg"] = agg_pool.tile([128, IC], F32, name="ps", tag="ps")
            at = a_pool.tile([128, len(js) * IC], F8, name="at", tag="at")
            a_eng().dma_start(
                out=at[:], in_=a0_in[row : row + 128, : len(js) * IC]
            )
            row += 128
            for t, j in enumerate(js):
                r, jl = j // JPR, j % JPR
                nc.tensor.matmul(
                    cur["agg"][:],
                    lhsT=stat_x[r][:, jl * 128 : (jl + 1) * 128],
                    rhs=at[:, t * IC : (t + 1) * IC],
                    start=(cur["k"] == 0),
                    stop=(cur["k"] == JT - 1),
                )
                cur["k"] += 1
            if cur["k"] == JT:
                linear(cur["agg"], w0_sb, emit0, cc)
                nc.gpsimd.collective_compute(
                    "AllGather",
                    mybir.AluOpType.bypass,
                    replica_groups=[list(range(NCORES))],
                    ins=[h_tb[cc][:]],
                    outs=[h_ag[cc][:]],
                )
                if cc > 0:
                    load_stats(cc - 1)  # after AG trigger: issues block on
                    # AG(cc-1) completion, never delaying this trigger
                cur["k"] = 0
        load_stats(NCH - 1)

        # ---- layer 1 (piece-outer: all 4 output chunks accumulate) ----
        agg1 = [
            ag1_pool.tile([128, IC], F32, name=f"ps1_{c}", tag=f"ps1_{c}")
            for c in range(NCH)
        ]

        def emit1(cc, it, lp):
            ht = h_pool.tile([128, D], F32, name="ht", tag="ht")
            nc.scalar.activation(ht[:], lp[:], relu)
            nc.scalar.dma_start(
                out=h_out[cc * IC + it * 128 : cc * IC + (it + 1) * 128, :],
                in_=ht[:],
            )

        row = 0
        kc = [0] * NCH  # per-output-chunk j progress
        for p, c, js in _l1_stream():
            at = a_pool.tile([128, len(js) * IC], F8, name="at", tag="at")
            a_eng().dma_start(out=at[:], in_=a1_in[row : row + 128, :])
            row += 128
            for t, j in enumerate(js):
                r = j // JPR
                tl = j % JPR - p * PJT
                nc.tensor.matmul(
                    agg1[c][:],
                    lhsT=sh[(r, p)][:, tl * 128 : (tl + 1) * 128],
                    rhs=at[:, t * IC : (t + 1) * IC],
                    start=(kc[c] == 0),
                    stop=(kc[c] == JT - 1),
                )
                kc[c] += 1
            if kc[c] == JT:
                linear(agg1[c], w1_sb, emit1, c)

    nc.finalize()
    return nc


def _tile_stat(X):
    return np.ascontiguousarray(
        X.reshape(NCORES, JPR, 128, D).transpose(0, 2, 1, 3)
        .reshape(NCORES * 128, ROWS)
    )


def _tile_a(aq4, stream, width_of):
    """aq4: [JT, 128, 2048] quantized A^T tiles. Lay out DMA-group rows
    [128, gj*w] (padded to 8*IC columns) following the stream order."""
    nrow = len(stream) * 128
    out = np.zeros((nrow, 8 * IC), dtype=aq4.dtype)
    for i, item in enumerate(stream):
        js, off, w = width_of(item)
        blk = aq4[js][:, :, off : off + w]          # [gj, 128, w]
        out[i * 128 : (i + 1) * 128, : len(js) * w] = (
            blk.transpose(1, 0, 2).reshape(128, len(js) * w)
        )
    return out


def shard_inputs(A_norm, X, W0, W1):
    """Host-side shard prep. Returns per-core input maps."""
    import ml_dtypes

    bf16 = ml_dtypes.bfloat16
    e4 = ml_dtypes.float8_e4m3

    x_t = _tile_stat(X).astype(bf16)
    w0b = (W0 / SCALE).astype(bf16)
    w1b = (W1 / SCALE).astype(bf16)
    s0, s1 = _l0_stream(), _l1_stream()

    in_maps = []
    for c in range(NCORES):
        a_tc = A_norm[c * ROWS : (c + 1) * ROWS, :].T  # [16384, 2048] view
        aq4 = (a_tc * np.float32(SCALE)).astype(e4).reshape(JT, 128, ROWS)
        a0 = _tile_a(aq4, s0, lambda it: (it[1], it[0] * IC, IC))
        a1 = _tile_a(aq4, s1, lambda it: (it[2], it[1] * IC, IC))
        in_maps.append({"a0": a0, "a1": a1, "x0": x_t, "w0": w0b, "w1": w1b})
    return in_maps


_CACHED = {}


def kernel(A_norm, X, W0, W1):
    A_norm = np.ascontiguousarray(A_norm, dtype=np.float32)
    X = np.ascontiguousarray(X, dtype=np.float32)
    W0 = np.ascontiguousarray(W0, dtype=np.float32)
    W1 = np.ascontiguousarray(W1, dtype=np.float32)

    from concourse.bass_utils import run_bass_kernel_spmd

    if PRECISION not in _CACHED:
        _CACHED[PRECISION] = build_gcn()
    nc = _CACHED[PRECISION]

    in_maps = shard_inputs(A_norm, X, W0, W1)
    res = run_bass_kernel_spmd(nc, in_maps, core_ids=list(range(NCORES)))
    return np.concatenate([res.results[c]["h_out"] for c in range(NCORES)], axis=0)
